# revision 124
# baseline (speedup 1.0000x reference)
"""Trainium2 Bass kernel for nn_EquivariantCrossAttention.

Sharding: batch*query rows (2*256=512) split across 8 cores (64 queries each,
cores 0-3 -> batch 0, cores 4-7 -> batch 1). k/v/a replicated per batch.

Per-core layout: feature-on-partition, (c,z) flattened on the free dim.
64 queries x 128 latents = 8192 free columns, processed in 16 chunks of 512.

Algebraic restructuring (validated vs reference in fp32 numpy):
  - All matmuls run in float32r (fp32 data, reduced-precision PE mode,
    1 cyc/row at N>=512 vs 4 for fp32). Producers write float32r tiles.
  - RFF dense biases folded into downstream weights (bqe->bq', bve->vb1').
  - LayerNorm gain/bias folded into the following matmul (W'=g*W, b'=bn@W+b).
  - vLN mean removed with one subtract; vLN rstd multiplied into h1 once
    (h1r) and commuted through the mW1/Wbm matmuls.
  - mixer-LN mean via rank-1 matmul fold (csmW2 x -mean); mixer rstd folded
    into the 8-row attention tile (attR) instead of the 128-row v2.
  - rstd = exp(-0.5*ln(var+eps)) so LN and softmax share the natural_log_exp
    activation-table set (hardware Rsqrt is forbidden; table swaps cost 2.7us).
  - FiLM: va*(1+gamma)+beta with the (1+bgam) part transposed host-style on
    the PE (amwT) and added inside the v1 PSUM accumulation via a tiled
    identity (eyeZ); Wbeta@mW1 folded on the host (Wbm).
  - LN statistics accumulated straight into multi-partition PSUM rows via
    one-hot selector matmuls (selS), read by ln_math in place.
  - v3 bias folded into the output projection bias via softmax-sum=1.
  - Softmax without max subtraction (logits are O(1) for this distribution);
    exp+normalize per quarter, in place on mid_all, inside the Ln/Exp window.
"""
import sys
import numpy as np

for _p in ("/opt/trn_rl_repo",):
    if _p not in sys.path:
        sys.path.insert(0, _p)

import concourse.bass as bass
import concourse.tile as tile
from concourse import bacc, mybir
from concourse.bass_utils import run_bass_kernel_spmd

FP = mybir.dt.float32
FR = mybir.dt.float32r
AF = mybir.ActivationFunctionType
OP = mybir.AluOpType
AX = mybir.AxisListType
ts = bass.ts

GELU_AF = AF.Gelu_apprx_tanh  # sim_test overrides (sim lacks gelu)

B, C, Z, D = 2, 256, 128, 3
H, NH, HH = 64, 8, 512
EPS = 1e-5
NCORE = 8
CPC = (B * C) // NCORE          # 64 queries per core
QC = 4                          # queries per chunk
CZ = QC * Z                     # 512 free columns per chunk
NCHUNK = CPC // QC              # 16
QSPLIT = 4                      # process h2 in quarters (SBUF)
CPQ = NCHUNK // QSPLIT          # 4 chunks per quarter
CZALL = CPC * Z                 # 8192


def _fp(ap):
    """Read a float32r AP as plain fp32 (same bits) for DVE/ACT consumers."""
    return ap.bitcast(FP)


# packed-constant layout: (name, base_row, nrows, ncols)
CPK_LAYOUT = [
    ("xp", 0, D, CPC + Z), ("aT", 0, H, Z), ("Bcat", 0, D, 128),
    ("qb", 0, 1, 128), ("bqc", 0, 128, 4), ("Wqv", 0, 128, 128),
    ("vW1", 64, H, H), ("vb1p", 0, H, 1), ("bcol", 0, 128, 8),
    ("maskS", 0, 128, 32), ("maskB", 64, NH, HH), ("maskT", 64, NH, 4 * 128),
    ("Wcat", 0, H, 5 * HH), ("brow", 0, 1, 5 * HH),
    ("mW1", 0, 128, 4 * HH), ("mW2", 0, 128, 4 * HH), ("Wo", 0, 128, 4 * HH),
    ("onec", 0, 128, 1), ("oner", 0, 1, CZ),
    ("selS", 0, 128, NCHUNK * NCHUNK), ("eyeZ", 0, 128, CZ),
]
CPK_COLS = {}
_c = 0
for _n, _r, _nr, _ncol in CPK_LAYOUT:
    CPK_COLS[_n] = (_r, _nr, _c, _ncol)
    _c += _ncol
CPK_NCOL = _c


def _bc(ap, outer):
    """[P,n] -> [P,outer,n] with stride-0 outer dim (broadcast over queries)."""
    return bass.AP(tensor=ap.tensor, offset=ap.offset,
                   ap=[ap.ap[0], [0, outer]] + list(ap.ap[1:]))


def _pbc(ap, nparts):
    """[1,n] -> [nparts,n] partition-broadcast AP (stride-0 partitions; DMA only)."""
    return bass.AP(tensor=ap.tensor, offset=ap.offset,
                   ap=[[0, nparts]] + list(ap.ap[1:]))


def _bc_inner(ap, inner):
    """[P,n] -> [P,n,inner] with stride-0 inner dim."""
    return bass.AP(tensor=ap.tensor, offset=ap.offset,
                   ap=list(ap.ap) + [[0, inner]])


def build_kernel():
    nc = bacc.Bacc("TRN2", target_bir_lowering=False, debug=False,
                   num_devices=NCORE)

    t = {}
    t["cpack"] = nc.dram_tensor("cpack", [128, CPK_NCOL], FR,
                                kind="ExternalInput").ap()
    t["out"] = nc.dram_tensor("out", [CPC, HH], FP, kind="ExternalOutput").ap()

    with tile.TileContext(nc) as tc:
        body(tc, t)
    nc.finalize()
    return nc


def body(tc, t):
    nc = tc.nc
    t = dict(t)
    t["scr_mr"] = nc.dram_tensor("scr_mr", [NCHUNK, 2, CZ], FP,
                                 kind="Internal").ap()
    t["scr_rm"] = nc.dram_tensor("scr_rm", [NCHUNK, CZ], FP, kind="Internal").ap()
    import contextlib
    stack = contextlib.ExitStack()
    P_const = stack.enter_context(tc.tile_pool(name="const", bufs=1))
    P_big = stack.enter_context(tc.tile_pool(name="big", bufs=1))

    cpk = P_const.tile([128, CPK_NCOL], FR, tag="cpk")
    nc.sync.dma_start(cpk[:], t["cpack"])

    S = {}
    for n, (r0, nr, c0, ncol) in CPK_COLS.items():
        S[n] = cpk[r0:r0 + nr, c0:c0 + ncol]
    Wcat = S["Wcat"].rearrange("p (k n) -> p k n", k=5)
    brow = S["brow"].rearrange("p (k n) -> p k n", k=5)
    S["xT"] = S["xp"][:, 0:CPC]
    S["pT"] = S["xp"][:, CPC:CPC + Z]
    S["Wq"], S["Wk"], S["Wv"] = Wcat[:, 0, :], Wcat[:, 1, :], Wcat[:, 2, :]
    S["Wgam"], S["Wbm"] = Wcat[:, 3, :], Wcat[:, 4, :]
    S["bqp"], S["bk"], S["bv"] = brow[:, 0, :], brow[:, 1, :], brow[:, 2, :]
    S["csmW2"], S["bopp"] = brow[:, 3, :], brow[:, 4, :]
    S["bgam1"], S["mb1pp"] = S["bcol"][:, 0:4], S["bcol"][:, 4:8]
    vW1_hi = S["vW1"]
    maskB_hi = S["maskB"]
    mW1_s = S["mW1"].rearrange("p (j n) -> p j n", j=4)
    mW2_s = S["mW2"].rearrange("p (j n) -> p j n", j=4)
    Wo_s = S["Wo"].rearrange("p (j n) -> p j n", j=4)
    ones_c = S["onec"]          # [128,1] float32r ones
    ones_r = S["oner"]          # [1,CZ]  float32r ones

    eps_c = P_const.tile([128, 1], FP)
    nc.vector.memset(eps_c[:], EPS)
    wj = P_const.tile([128, CZ], FP)
    nc.vector.memset(wj[:], 0.0)

    def _warm(pool, n, ncols=CZ):
        """Throwaway fp32 matmuls: keep the PE HAM clock-gate at 8/8.
        The gate tracks busy FRACTION per 3.4us window, so the filler must
        contribute real busy time (N=512 fp32 ~850 ns warm each)."""
        for _r in range(n):
            jp = pool.tile([128, CZ], FP, tag="junk")
            nc.tensor.matmul(jp[:, 0:ncols], wj[:, 0:128], wj[:, 0:ncols],
                             start=True, stop=True)

    # dense burst while the cpack DMA streams in: enter PRE at 2.4 GHz
    with tc.tile_pool(name="warm_ps", bufs=1, space="PSUM") as WP:
        _warm(WP, 16)

    # persistent buffers
    # mid_all: rows 0-63 h1, rows 64-71 logits->attention (in place)
    mid_all = P_big.tile([128, CZALL], FR)
    h1_all = mid_all  # h1 = mid_all[0:64]
    y_all = P_big.tile([128, 4, CPC], FR)
    MvRv = P_big.tile([NCHUNK, 2, CZ], FP)  # vLN mean | rstd, interleaved
    nMq = P_big.tile([CPQ, CZ], FR)
    RmQ = P_big.tile([CPQ, CZ], FP)
    nMm1 = P_big.tile([1, CPQ * CZ], FR)   # one quarter's negated means, row form
    esum_all = P_big.tile([64 + NH, CPC], FP)  # softmax 1/sum, rows 64-71
    kv_s = P_big.tile([128, 4, Z], FP)
    va_s = P_big.tile([128, 4, Z], FP)
    amwT_s = P_big.tile([128, HH], FR)     # [z, f] = (va*(1+bgam)) @ mW1, transposed
    bqkT_s = P_big.tile([Z, NH], FR)       # [z, h] = 0.125 * sum_{f in h} bq[f]k[f,z]

    # ---------------- PRE: k, va, A0, amwT ----------------
    with tc.tile_pool(name="pre_ps", bufs=2, space="PSUM") as PP, \
         tc.tile_pool(name="pre_sb", bufs=1) as PSB:
        A0_s = PSB.tile([128, 4, Z], FR)
        # kv_s carries the 1/sqrt(H)=0.125 attention scale
        for dst_s, W_n, b_n, scl in [(kv_s, "Wk", "bk", 0.125),
                                     (va_s, "Wv", "bv", 1.0)]:
            for tt in range(4):
                ps = PP.tile([128, Z], FP, tag="kv")
                nc.tensor.matmul(ps[:], S[W_n][:, ts(tt, 128)],
                                 S["aT"][:], start=True, stop=False)
                nc.tensor.matmul(ps[:], S[b_n][:, ts(tt, 128)],
                                 ones_r[:, 0:Z], start=False, stop=True)
                nc.scalar.activation(dst_s[:, tt, :], ps[:], AF.Copy,
                                     scale=scl)
        for tt in range(4):
            nc.scalar.mul(A0_s[:, tt, :], va_s[:, tt, :],
                          _fp(S["bgam1"][:, tt:tt + 1]))
        # amwT[z, f] = sum_g A0[g, z] * mW1[g, f], accumulated over 4 g-groups
        aps = PP.tile([128, HH], FP, tag="amwT")
        for j in range(4):
            nc.tensor.matmul(aps[:], A0_s[:, j, :], mW1_s[:, j, :],
                             start=(j == 0), stop=(j == 3))
        nc.scalar.copy(amwT_s[:], aps[:])
        # bqkT[z, h] = 0.125 * sum_f bq[f] k[f,z] [head(f)==h] (q-bias logits)
        bqk_ps = PP.tile([Z, NH], FP, tag="bqk")
        for tt in range(4):
            ek0 = PSB.tile([128, Z], FR, tag="ek0")
            nc.scalar.mul(ek0[:], kv_s[:, tt, :], _fp(S["bqc"][:, tt:tt + 1]))
            nc.tensor.matmul(bqk_ps[:], ek0[:], S["maskS"][:, ts(tt, NH)],
                             start=(tt == 0), stop=(tt == 3))
        nc.scalar.copy(bqkT_s[:], bqk_ps[:])

    # ---------------- B0/B1 under the ie_all pool ----------------
    with tc.tile_pool(name="iep", bufs=1) as ie_pool:
        ie_all = ie_pool.tile([128, CZALL], FR)
        # ---- B0: inv -> RFF -> ie (sin) ----
        with tc.tile_pool(name="b0_ps", bufs=2, space="PSUM") as PP, \
             tc.tile_pool(name="b0_jk", bufs=1, space="PSUM") as JP0, \
             tc.tile_pool(name="b0_sb", bufs=3) as SB:
            RC = 12582912.0  # 1.5 * 2^23: fp32 add rounds to nearest integer
            for i in range(NCHUNK):
                _warm(JP0, 3, ncols=320)
                cols = ts(i, CZ)
                inv = SB.tile([D, QC, Z], FR, tag="inv")
                nc.vector.tensor_sub(
                    inv[:], _bc_inner(_fp(S["xT"])[:, ts(i, QC)], Z),
                    _bc(_fp(S["pT"])[:, :], QC))
                # rows: [m_q, m_q+0.25, m_v, m_v+0.25] (unit-period phases)
                mm = PP.tile([128, CZ], FP, tag="mm")
                nc.tensor.matmul(mm[:], S["Bcat"][:], inv[:], start=True,
                                 stop=False)
                nc.tensor.matmul(mm[:], S["qb"][:], ones_r[:], start=False,
                                 stop=True)
                r1 = SB.tile([128, CZ], FP, tag="r1")
                nc.scalar.activation(r1[:], mm[:], AF.Copy, bias=RC)
                fr = SB.tile([128, CZ], FP, tag="fr")
                nc.vector.scalar_tensor_tensor(fr[:], r1[:], RC, mm[:],
                                               op0=OP.subtract,
                                               op1=OP.subtract)
                F = SB.tile([128, CZ], FR, tag="F")
                nc.scalar.activation(F[:], fr[:], AF.Sin,
                                     scale=float(2 * np.pi))
                ieps = PP.tile([128, CZ], FP, tag="ieps")
                nc.tensor.matmul(ieps[:], S["Wqv"][:], F[:],
                                 start=True, stop=True)
                nc.vector.tensor_copy(ie_all[:, cols], ieps[:])

        # ---- B1: q/logits, h1, vLN stats into PSUM (gelu) ----
        with tc.tile_pool(name="b1_st", bufs=1, space="PSUM") as PPS1:
            SvP = PPS1.tile([NCHUNK, CZ], FP)
            QvP = PPS1.tile([NCHUNK, CZ], FP)
            with tc.tile_pool(name="b1_ps", bufs=1, space="PSUM") as PP, \
                 tc.tile_pool(name="b1_qps", bufs=3, space="PSUM") as PPQ, \
                 tc.tile_pool(name="b1_jk", bufs=1, space="PSUM") as JP1, \
                 tc.tile_pool(name="b1_ek", bufs=8) as SBE, \
                 tc.tile_pool(name="b1_sb", bufs=2) as SB:
                for i in range(NCHUNK):
                    _warm(JP1, 2, ncols=224)
                    cols = ts(i, CZ)
                    # h1 path first: its gelu/square run on ACT while the PE
                    # works through the q-path matmuls below
                    h1ps = PP.tile([H, CZ], FP, tag="h1ps")
                    nc.tensor.matmul(h1ps[:], vW1_hi[:], ie_all[64:128, cols],
                                     start=True, stop=True)
                    nc.scalar.activation(h1_all[0:64, cols], h1ps[:], GELU_AF,
                                         bias=_fp(S["vb1p"])[:])
                    sq = SB.tile([H, CZ], FR, tag="sq")
                    nc.scalar.square(sq[:], _fp(h1_all[0:64, cols]))
                    qpss = []
                    for tt in range(4):
                        qps = PPQ.tile([128, CZ], FP, tag="qps")
                        nc.tensor.matmul(qps[:], S["Wq"][:, ts(tt, 128)],
                                         ie_all[0:64, cols],
                                         start=True, stop=True)
                        qpss.append(qps)
                    eks = []
                    for tt in range(4):
                        ek = SBE.tile([128, CZ], FR, tag="ek")
                        nc.vector.tensor_mul(ek[:], qpss[tt][:],
                                             _bc(kv_s[:, tt, :], QC))
                        eks.append(ek)
                    lps = PP.tile([NH, CZ], FP, tag="lps")
                    for tt in range(4):
                        nc.tensor.matmul(lps[:], S["maskS"][:, ts(tt, NH)],
                                         eks[tt][:], start=(tt == 0),
                                         stop=False)
                    nc.tensor.matmul(lps[:], bqkT_s[:], S["eyeZ"][:],
                                     start=False, stop=True)
                    nc.scalar.copy(mid_all[64:64 + NH, cols], lps[:])
                    sel = S["selS"][0:64, ts(i, NCHUNK)]
                    nc.tensor.matmul(SvP[:], sel, h1_all[0:64, cols],
                                     start=(i == 0), stop=(i == NCHUNK - 1))
                    nc.tensor.matmul(QvP[:], sel, sq[:],
                                     start=(i == 0), stop=(i == NCHUNK - 1))

            # ---- C1: vLN rstd (ln/exp); stats read from PSUM in place ----
            ln_math(nc, slice(0, NCHUNK), SvP, QvP, MvRv[:, 0, :], float(H),
                    False, MvRv[:, 1, :], eps_c)
            nc.sync.dma_start(t["scr_mr"], MvRv[:])

    # ---------------- quarters: B2 (gelu) -> ln+softmax -> D -------------
    h2_pool = stack.enter_context(tc.tile_pool(name="h2p", bufs=1))
    h2_q = h2_pool.tile([128, 4, CPQ * CZ], FR)
    P_bc = stack.enter_context(tc.tile_pool(name="bcast", bufs=8))

    def prefetch_mr(qq):
        """Broadcast each chunk's vLN (mean|rstd) row pair to H rows."""
        pf = []
        for ii in range(CPQ):
            i = qq * CPQ + ii
            mr = P_bc.tile([H, 2, CZ], FP, tag="mr")
            src = t["scr_mr"][i, :, :]
            nc.sync.dma_start(mr[:], bass.AP(tensor=src.tensor,
                                             offset=src.offset,
                                             ap=[[0, H]] + list(src.ap)))
            pf.append(mr)
        return pf

    pf_cur = prefetch_mr(0)
    for qq in range(QSPLIT):
        with tc.tile_pool(name="b2_st", bufs=1, space="PSUM") as PPS:
            SmP = PPS.tile([CPQ, CZ], FP, tag="SmP")
            QmP = PPS.tile([CPQ, CZ], FP, tag="QmP")
            with tc.tile_pool(name="qb_jk", bufs=1, space="PSUM") as JPQ:
                _warm(JPQ, 3)
            with tc.tile_pool(name="b2_pg", bufs=2, space="PSUM") as PPG, \
                 tc.tile_pool(name="b2_v1", bufs=4, space="PSUM") as PPV, \
                 tc.tile_pool(name="b2_s4", bufs=4) as SB4, \
                 tc.tile_pool(name="b2_sb", bufs=2) as SB, \
                 tc.tile_pool(name="b2_g4", bufs=7) as SBG:
                h1rs = []
                for ii in range(CPQ):
                    cols = ts(qq * CPQ + ii, CZ)
                    h1c = SB.tile([H, CZ], FP, tag="h1c")
                    nc.vector.tensor_sub(h1c[:], _fp(h1_all[0:64, cols]),
                                         pf_cur[ii][:, 0, :])
                    h1r = SB4.tile([H, CZ], FR, tag="h1r")
                    nc.vector.tensor_mul(h1r[:], h1c[:], pf_cur[ii][:, 1, :])
                    h1rs.append(h1r)
                for ii in range(CPQ):
                    qcols = ts(ii, CZ)
                    h1r = h1rs[ii]
                    Gs = []
                    for tt in range(4):
                        pg = PPG.tile([128, CZ], FP, tag="pg")
                        nc.tensor.matmul(pg[:], S["Wgam"][:, ts(tt, 128)],
                                         h1r[:], start=True, stop=True)
                        G = SBG.tile([128, CZ], FR, tag="G")
                        nc.vector.tensor_mul(G[:], _bc(va_s[:, tt, :], QC),
                                             pg[:])
                        Gs.append(G)
                    for dst in range(4):
                        v1p = PPV.tile([128, CZ], FP, tag="v1p")
                        for tt in range(4):
                            nc.tensor.matmul(v1p[:],
                                             mW1_s[:, tt, ts(dst, 128)],
                                             Gs[tt][:], start=(tt == 0),
                                             stop=False)
                        nc.tensor.matmul(v1p[:], S["Wbm"][:, ts(dst, 128)],
                                         h1r[:], start=False, stop=False)
                        nc.tensor.matmul(v1p[:], amwT_s[:, ts(dst, 128)],
                                         S["eyeZ"][:], start=False, stop=True)
                        nc.scalar.activation(h2_q[:, dst, qcols], v1p[:],
                                             GELU_AF,
                                             bias=_fp(S["mb1pp"])[:,
                                                                  dst:dst + 1])
                    # stats after the chunk's gelus
                    sel = S["selS"][:, ts(ii, NCHUNK)][:, 0:CPQ]
                    for dst in range(4):
                        nc.tensor.matmul(SmP[:], sel, h2_q[:, dst, qcols],
                                         start=(ii == 0 and dst == 0),
                                         stop=(ii == CPQ - 1 and dst == 3))
                        sq2 = SB.tile([128, CZ], FR, tag="sq2")
                        nc.scalar.square(sq2[:], _fp(h2_q[:, dst, qcols]))
                        nc.tensor.matmul(QmP[:], sel, sq2[:],
                                         start=(ii == 0 and dst == 0),
                                         stop=(ii == CPQ - 1 and dst == 3))

            # ---- mixer LN stats + per-quarter softmax (ln/exp table) ----
            # prefetch next quarter's mean/rstd rows ahead of the ln chain
            # so they don't queue behind it on the sync engine
            pf_next = prefetch_mr(qq + 1) if qq + 1 < QSPLIT else None
            qall = ts(qq, CPQ * CZ)
            ln_math(nc, slice(0, CPQ), SmP, QmP, nMq, float(HH), True, RmQ,
                    eps_c, mt_fr=True)
            nc.sync.dma_start(nMm1[:, :], nMq[0:CPQ, :])
            nc.sync.dma_start(t["scr_rm"][qq * CPQ:(qq + 1) * CPQ, :],
                              RmQ[0:CPQ, :])
            # unnormalized softmax: attention stays exp(logits); the 1/sum
            # is applied once to y_all right before the output projection
            attq = mid_all[64:64 + NH, qall]
            nc.scalar.activation(attq, _fp(attq), AF.Exp)
            esq = esum_all[64:64 + NH, ts(qq, CPQ * QC)]
            nc.vector.reduce_sum(
                esq, _fp(attq).rearrange("p (c z) -> p c z", z=Z), axis=AX.X)
            nc.vector.reciprocal(esq, esq)
            with tc.tile_pool(name="ln_jk", bufs=1, space="PSUM") as JPL:
                _warm(JPL, 3)

        # ---- D: v2, rank-1 mean fix, attention apply ----
        with tc.tile_pool(name="d_v2", bufs=5, space="PSUM") as PPV2, \
             tc.tile_pool(name="d_ab", bufs=2, space="PSUM") as PPA, \
             tc.tile_pool(name="d_jk", bufs=1, space="PSUM") as JPD, \
             tc.tile_pool(name="d_s4", bufs=4) as SD4, \
             tc.tile_pool(name="d_r2", bufs=4) as SDR, \
             tc.tile_pool(name="d_sb", bufs=2) as SB:
            rmss = []
            for ii in range(CPQ):
                i = qq * CPQ + ii
                rms8t = SDR.tile([64 + NH, CZ], FP, tag="rms8")
                nc.sync.dma_start(rms8t[64:64 + NH, :],
                                  _pbc(t["scr_rm"][i:i + 1, :], NH))
                rmss.append(rms8t)
            for ii in range(CPQ):
                _warm(JPD, 1, ncols=256)
                i = qq * CPQ + ii
                cols = ts(i, CZ)
                qcols = ts(ii, CZ)
                # mixer-LN rstd folded into the 8-row attention tile
                attRt = SD4.tile([64 + NH, CZ], FR, tag="attR")
                attR = attRt[64:64 + NH, :]
                nc.vector.tensor_mul(attR, _fp(mid_all[64:64 + NH, cols]),
                                     rmss[ii][64:64 + NH, :])
                for dst in range(4):
                    v2p = PPV2.tile([128, CZ], FP, tag="v2p")
                    for j in range(4):
                        nc.tensor.matmul(v2p[:], mW2_s[:, j, ts(dst, 128)],
                                         h2_q[:, j, qcols],
                                         start=(j == 0), stop=False)
                    nc.tensor.matmul(v2p[:], S["csmW2"][:, ts(dst, 128)],
                                     nMm1[:, qcols], start=False, stop=True)
                    ab = PPA.tile([128, CZ], FP, tag="ab")
                    nc.tensor.matmul(ab[:], maskB_hi[:, ts(dst, 128)],
                                     attR, start=True, stop=True)
                    abs_ = SB.tile([128, CZ], FP, tag="abs")
                    nc.scalar.copy(abs_[:], ab[:])
                    yp = SB.tile([128, QC, Z], FP, tag="yp")
                    nc.vector.tensor_mul(
                        yp[:], abs_[:].rearrange("p (c z) -> p c z", z=Z),
                        v2p[:].rearrange("p (c z) -> p c z", z=Z))
                    with nc.allow_low_precision(reason="fp32r y"):
                        nc.vector.reduce_sum(
                            y_all[:, dst, i * QC:(i + 1) * QC],
                            yp[:], axis=AX.X)
        pf_cur = pf_next

    # ---------------- OUT ----------------
    with tc.tile_pool(name="o_ps", bufs=1, space="PSUM") as PP, \
         tc.tile_pool(name="o_sb", bufs=1) as SB:
        # per-feature gather of the deferred softmax 1/sum, then normalize
        esY = PP.tile([128, 4 * CPC], FP)
        for tt in range(4):
            nc.tensor.matmul(esY[:, ts(tt, CPC)],
                             _fp(S["maskT"])[:, ts(tt, 128)],
                             esum_all[64:64 + NH, :], start=True, stop=True)
        y_n = SB.tile([128, 4, CPC], FR)
        nc.vector.tensor_mul(y_n[:], _fp(y_all[:]),
                             esY[:].rearrange("p (t c) -> p t c", t=4))
        ops = PP.tile([CPC, HH], FP)
        for j in range(4):
            nc.tensor.matmul(ops[:], y_n[:, j, :], Wo_s[:, j, :],
                             start=(j == 0), stop=False)
        nc.tensor.matmul(ops[:], ones_r[:, 0:CPC], S["bopp"][:],
                         start=False, stop=True)
        osb = SB.tile([CPC, HH], FP)
        nc.scalar.copy(osb[:], ops[:])
        nc.sync.dma_start(t["out"], osb[:])
    stack.close()


def ln_math(nc, rows, St, Qt, Mt, n, negate_mean, Rt, eps_c, mt_fr=False):
    # St/Qt may live in PSUM (DVE reads at most one PSUM input per op).
    # Mt = (+-)mean; Rt staged as scratch for S^2/n; Qt consumed in place.
    mt_rd = (lambda ap: _fp(ap)) if mt_fr else (lambda ap: ap)
    sgn = -1.0 if negate_mean else 1.0
    nc.vector.tensor_scalar_mul(Mt[rows, :], St[rows, :], sgn / n)
    nc.vector.tensor_mul(Rt[rows, :], St[rows, :], mt_rd(Mt[rows, :]))
    if negate_mean:
        nc.vector.tensor_add(Qt[rows, :], Qt[rows, :], Rt[rows, :])
    else:
        nc.vector.tensor_sub(Qt[rows, :], Qt[rows, :], Rt[rows, :])
    nc.scalar.activation(Qt[rows, :], Qt[rows, :], AF.Ln,
                         scale=1.0 / n, bias=eps_c[rows, :])
    nc.scalar.activation(Rt[rows, :], Qt[rows, :], AF.Exp, scale=-0.5)


# ======================= host side =======================
_CACHE = {}


def _pack_consts(P):
    A = np.zeros((128, CPK_NCOL), np.float32)
    for n, (r0, nr, c0, ncol) in CPK_COLS.items():
        if n in ("xp", "aT"):
            continue
        v = P[n]
        assert v.shape == (nr, ncol), (n, v.shape, nr, ncol)
        A[r0:r0 + nr, c0:c0 + ncol] = v
    return A


def _host_prep(inp):
    g = {k: np.ascontiguousarray(np.asarray(v, np.float32)) for k, v in inp.items()}
    P = {}
    P["Bcat"] = np.concatenate([g["B_q"], g["B_q"], g["B_v"], g["B_v"]], 1)
    qb = np.zeros((1, 128), np.float32)
    qb[0, 32:64] = 0.25
    qb[0, 96:128] = 0.25
    P["qb"] = qb
    Wqv = np.zeros((128, 128), np.float32)
    Wqv[0:64, 0:64] = -np.concatenate([g["Wqe"][:32], g["Wqe"][32:]], 0)
    Wqv[64:128, 64:128] = -np.concatenate([g["Wve"][:32], g["Wve"][32:]], 0)
    P["Wqv"] = Wqv
    bqp = (g["bqe"] @ g["Wq"] + g["bq"])[None, :]
    P["bqc"] = np.ascontiguousarray(bqp.reshape(4, 128).T)  # 0.125 is in kv_s
    P["vW1"] = g["vW1"]
    P["vb1p"] = (g["bve"] @ g["vW1"] + g["vb1"])[:, None]
    vW2p = g["vg"][:, None] * g["vW2"]
    vb2p = g["vbn"] @ g["vW2"] + g["vb2"]
    Wgam = vW2p[:, :HH]
    Wbeta, bbeta = vW2p[:, HH:], vb2p[HH:]
    bgam1 = np.ascontiguousarray((1.0 + vb2p[:HH]).reshape(4, 128).T)
    P["mW1"] = g["mW1"]
    Wbm = Wbeta @ g["mW1"]
    mb1pp = np.ascontiguousarray(
        (bbeta @ g["mW1"] + g["mb1"]).reshape(4, 128).T)
    mW2p = g["mg"][:, None] * g["mW2"]
    mb2p = g["mbn"] @ g["mW2"] + g["mb2"]
    P["mW2"] = mW2p
    csmW2 = mW2p.sum(0)[None, :]
    P["Wo"] = g["Wo"]
    bopp = (mb2p @ g["Wo"] + g["bo"])[None, :]
    P["Wcat"] = np.concatenate([g["Wq"], g["Wk"], g["Wv"], Wgam, Wbm], 1)
    P["brow"] = np.concatenate([bqp, g["bk"][None, :], g["bv"][None, :],
                                csmW2, bopp], 1)
    P["bcol"] = np.concatenate([bgam1, mb1pp], 1)
    for wn in ("mW1", "mW2", "Wo"):
        P[wn] = np.ascontiguousarray(
            P[wn].reshape(4, 128, HH).transpose(1, 0, 2).reshape(128, 4 * HH))
    P["onec"] = np.ones((128, 1), np.float32)
    P["oner"] = np.ones((1, CZ), np.float32)
    selS = np.zeros((128, NCHUNK, NCHUNK), np.float32)
    for i in range(NCHUNK):
        selS[:, i, i] = 1.0
    P["selS"] = np.ascontiguousarray(selS.reshape(128, NCHUNK * NCHUNK))
    P["eyeZ"] = np.ascontiguousarray(np.tile(np.eye(Z, dtype=np.float32),
                                             (1, QC)))
    mS = np.zeros((128, 4, NH), np.float32)
    for tt in range(4):
        for p in range(128):
            mS[p, tt, 2 * tt + p // 64] = 1.0
    P["maskS"] = np.ascontiguousarray(mS.reshape(128, 32))
    P["maskB"] = np.zeros((NH, HH), np.float32)
    for h in range(NH):
        P["maskB"][h, h * H:(h + 1) * H] = 1.0
    mT = np.zeros((NH, 4, 128), np.float32)
    for tt in range(4):
        for p in range(128):
            mT[2 * tt + p // 64, tt, p] = 1.0
    P["maskT"] = np.ascontiguousarray(mT.reshape(NH, 4 * 128))
    return P, g


def make_in_maps(P, g):
    base = _pack_consts(P)
    xT_full = np.ascontiguousarray(g["inputs"].reshape(B * C, D).T)
    in_maps = []
    for core in range(NCORE):
        b = core // (NCORE // B)
        A = base.copy()
        r0, nr, c0, ncol = CPK_COLS["xp"]
        A[r0:r0 + nr, c0:c0 + ncol] = np.concatenate(
            [xT_full[:, core * CPC:(core + 1) * CPC], g["p"][b].T], 1)
        r0, nr, c0, ncol = CPK_COLS["aT"]
        A[r0:r0 + nr, c0:c0 + ncol] = g["a"][b].T
        in_maps.append({"cpack": A})
    return in_maps


def kernel(**inputs):
    P, g = _host_prep(inputs)
    if "nc" not in _CACHE:
        _CACHE["nc"] = build_kernel()
    nc = _CACHE["nc"]
    in_maps = make_in_maps(P, g)
    res = run_bass_kernel_spmd(nc, in_maps, core_ids=list(range(NCORE)))
    outs = [res.results[i]["out"] for i in range(NCORE)]
    return np.concatenate(outs, 0).reshape(B, C, HH).astype(np.float32)


if __name__ == "__main__":
    import reference
    inp = {k: np.asarray(v) for k, v in reference.setup_inputs().items()}
    got = kernel(**inp)
    exp = np.asarray(reference.reference(**reference.setup_inputs()))
    err = np.abs(got - exp)
    scale = float(np.sqrt((exp ** 2).mean()))
    print("max abs err:", err.max(), " scaled:", err.max() / scale)


# revision 126
# speedup vs baseline: 1.0094x; 1.0094x over previous
"""Trainium2 Bass kernel for nn_EquivariantCrossAttention.

Sharding: batch*query rows (2*256=512) split across 8 cores (64 queries each,
cores 0-3 -> batch 0, cores 4-7 -> batch 1). k/v/a replicated per batch.

Per-core layout: feature-on-partition, (c,z) flattened on the free dim.
64 queries x 128 latents = 8192 free columns, processed in 16 chunks of 512.

Algebraic restructuring (validated vs reference in fp32 numpy):
  - All matmuls run in float32r (fp32 data, reduced-precision PE mode,
    1 cyc/row at N>=512 vs 4 for fp32). Producers write float32r tiles.
  - RFF dense biases folded into downstream weights (bqe->bq', bve->vb1').
  - LayerNorm gain/bias folded into the following matmul (W'=g*W, b'=bn@W+b).
  - vLN mean removed with one subtract; vLN rstd multiplied into h1 once
    (h1r) and commuted through the mW1/Wbm matmuls.
  - mixer-LN mean via rank-1 matmul fold (csmW2 x -mean); mixer rstd folded
    into the 8-row attention tile (attR) instead of the 128-row v2.
  - rstd = exp(-0.5*ln(var+eps)) so LN and softmax share the natural_log_exp
    activation-table set (hardware Rsqrt is forbidden; table swaps cost 2.7us).
  - FiLM: va*(1+gamma)+beta with the (1+bgam) part transposed host-style on
    the PE (amwT) and added inside the v1 PSUM accumulation via a tiled
    identity (eyeZ); Wbeta@mW1 folded on the host (Wbm).
  - LN statistics accumulated straight into multi-partition PSUM rows via
    one-hot selector matmuls (selS), read by ln_math in place.
  - v3 bias folded into the output projection bias via softmax-sum=1.
  - Softmax without max subtraction (logits are O(1) for this distribution);
    exp+normalize per quarter, in place on mid_all, inside the Ln/Exp window.
"""
import sys
import numpy as np

for _p in ("/opt/trn_rl_repo",):
    if _p not in sys.path:
        sys.path.insert(0, _p)

import concourse.bass as bass
import concourse.tile as tile
from concourse import bacc, mybir
from concourse.bass_utils import run_bass_kernel_spmd

FP = mybir.dt.float32
FR = mybir.dt.float32r
AF = mybir.ActivationFunctionType
OP = mybir.AluOpType
AX = mybir.AxisListType
ts = bass.ts

GELU_AF = AF.Gelu_apprx_tanh  # sim_test overrides (sim lacks gelu)

B, C, Z, D = 2, 256, 128, 3
H, NH, HH = 64, 8, 512
EPS = 1e-5
NCORE = 8
CPC = (B * C) // NCORE          # 64 queries per core
QC = 4                          # queries per chunk
CZ = QC * Z                     # 512 free columns per chunk
NCHUNK = CPC // QC              # 16
QSPLIT = 4                      # process h2 in quarters (SBUF)
CPQ = NCHUNK // QSPLIT          # 4 chunks per quarter
CZALL = CPC * Z                 # 8192


def _fp(ap):
    """Read a float32r AP as plain fp32 (same bits) for DVE/ACT consumers."""
    return ap.bitcast(FP)


# packed-constant layout: (name, base_row, nrows, ncols)
CPK_LAYOUT = [
    ("xp", 0, D, CPC + Z), ("aT", 0, H, Z), ("Bcat", 0, D, 128),
    ("qb", 0, 1, 128), ("bqc", 0, 128, 4), ("Wqv", 0, 128, 128),
    ("vW1", 64, H, H), ("vb1p", 0, H, 1), ("bcol", 0, 128, 8),
    ("maskS", 0, 128, 32), ("maskB", 64, NH, HH), ("maskT", 64, NH, 4 * 128),
    ("Wcat", 0, H, 5 * HH), ("brow", 0, 1, 5 * HH),
    ("mW1", 0, 128, 4 * HH), ("mW2", 0, 128, 4 * HH), ("Wo", 0, 128, 4 * HH),
    ("onec", 0, 128, 1), ("oner", 0, 1, CZ),
    ("selS", 0, 128, NCHUNK * NCHUNK), ("eyeZ", 0, 128, CZ),
]
CPK_COLS = {}
_c = 0
for _n, _r, _nr, _ncol in CPK_LAYOUT:
    CPK_COLS[_n] = (_r, _nr, _c, _ncol)
    _c += _ncol
CPK_NCOL = _c


def _bc(ap, outer):
    """[P,n] -> [P,outer,n] with stride-0 outer dim (broadcast over queries)."""
    return bass.AP(tensor=ap.tensor, offset=ap.offset,
                   ap=[ap.ap[0], [0, outer]] + list(ap.ap[1:]))


def _pbc(ap, nparts):
    """[1,n] -> [nparts,n] partition-broadcast AP (stride-0 partitions; DMA only)."""
    return bass.AP(tensor=ap.tensor, offset=ap.offset,
                   ap=[[0, nparts]] + list(ap.ap[1:]))


def _bc_inner(ap, inner):
    """[P,n] -> [P,n,inner] with stride-0 inner dim."""
    return bass.AP(tensor=ap.tensor, offset=ap.offset,
                   ap=list(ap.ap) + [[0, inner]])


def build_kernel():
    nc = bacc.Bacc("TRN2", target_bir_lowering=False, debug=False,
                   num_devices=NCORE)

    t = {}
    t["cpack"] = nc.dram_tensor("cpack", [128, CPK_NCOL], FR,
                                kind="ExternalInput").ap()
    t["out"] = nc.dram_tensor("out", [CPC, HH], FP, kind="ExternalOutput").ap()

    with tile.TileContext(nc) as tc:
        body(tc, t)
    nc.finalize()
    return nc


def body(tc, t):
    nc = tc.nc
    t = dict(t)
    t["scr_mr"] = nc.dram_tensor("scr_mr", [NCHUNK, 2, CZ], FP,
                                 kind="Internal").ap()
    t["scr_rm"] = nc.dram_tensor("scr_rm", [NCHUNK, CZ], FP, kind="Internal").ap()
    import contextlib
    stack = contextlib.ExitStack()
    P_const = stack.enter_context(tc.tile_pool(name="const", bufs=1))
    P_big = stack.enter_context(tc.tile_pool(name="big", bufs=1))

    cpk = P_const.tile([128, CPK_NCOL], FR, tag="cpk")
    nc.sync.dma_start(cpk[:], t["cpack"])

    S = {}
    for n, (r0, nr, c0, ncol) in CPK_COLS.items():
        S[n] = cpk[r0:r0 + nr, c0:c0 + ncol]
    Wcat = S["Wcat"].rearrange("p (k n) -> p k n", k=5)
    brow = S["brow"].rearrange("p (k n) -> p k n", k=5)
    S["xT"] = S["xp"][:, 0:CPC]
    S["pT"] = S["xp"][:, CPC:CPC + Z]
    S["Wq"], S["Wk"], S["Wv"] = Wcat[:, 0, :], Wcat[:, 1, :], Wcat[:, 2, :]
    S["Wgam"], S["Wbm"] = Wcat[:, 3, :], Wcat[:, 4, :]
    S["bqp"], S["bk"], S["bv"] = brow[:, 0, :], brow[:, 1, :], brow[:, 2, :]
    S["csmW2"], S["bopp"] = brow[:, 3, :], brow[:, 4, :]
    S["bgam1"], S["mb1pp"] = S["bcol"][:, 0:4], S["bcol"][:, 4:8]
    vW1_hi = S["vW1"]
    maskB_hi = S["maskB"]
    mW1_s = S["mW1"].rearrange("p (j n) -> p j n", j=4)
    mW2_s = S["mW2"].rearrange("p (j n) -> p j n", j=4)
    Wo_s = S["Wo"].rearrange("p (j n) -> p j n", j=4)
    ones_c = S["onec"]          # [128,1] float32r ones
    ones_r = S["oner"]          # [1,CZ]  float32r ones

    eps_c = P_const.tile([128, 1], FP)
    nc.vector.memset(eps_c[:], EPS)
    wj = P_const.tile([128, CZ], FP)
    nc.vector.memset(wj[:], 0.0)

    def _warm(pool, n, ncols=CZ):
        """Throwaway fp32 matmuls: keep the PE HAM clock-gate at 8/8.
        The gate tracks busy FRACTION per 3.4us window, so the filler must
        contribute real busy time (N=512 fp32 ~850 ns warm each)."""
        for _r in range(n):
            jp = pool.tile([128, CZ], FP, tag="junk")
            nc.tensor.matmul(jp[:, 0:ncols], wj[:, 0:128], wj[:, 0:ncols],
                             start=True, stop=True)

    # dense burst while the cpack DMA streams in: enter PRE at 2.4 GHz
    with tc.tile_pool(name="warm_ps", bufs=1, space="PSUM") as WP:
        _warm(WP, 16)

    # persistent buffers
    # mid_all: rows 0-63 h1, rows 64-71 logits->attention (in place)
    mid_all = P_big.tile([128, CZALL], FR)
    h1_all = mid_all  # h1 = mid_all[0:64]
    y_all = P_big.tile([128, 4, CPC], FR)
    MvRv = P_big.tile([NCHUNK, 2, CZ], FP)  # vLN mean | rstd, interleaved
    nMq = P_big.tile([CPQ, CZ], FR)
    RmQ = P_big.tile([CPQ, CZ], FP)
    nMm1 = P_big.tile([1, CPQ * CZ], FR)   # one quarter's negated means, row form
    esum_all = P_big.tile([64 + NH, CPC], FP)  # softmax 1/sum, rows 64-71
    kv_s = P_big.tile([128, 4, Z], FP)
    va_s = P_big.tile([128, 4, Z], FP)
    amwT_s = P_big.tile([128, HH], FR)     # [z, f] = (va*(1+bgam)) @ mW1, transposed
    bqkT_s = P_big.tile([Z, NH], FR)       # [z, h] = 0.125 * sum_{f in h} bq[f]k[f,z]

    # ---------------- PRE: k, va, A0, amwT ----------------
    with tc.tile_pool(name="pre_ps", bufs=2, space="PSUM") as PP, \
         tc.tile_pool(name="pre_sb", bufs=1) as PSB:
        A0_s = PSB.tile([128, 4, Z], FR)
        # kv_s carries the 1/sqrt(H)=0.125 attention scale
        for dst_s, W_n, b_n, scl in [(kv_s, "Wk", "bk", 0.125),
                                     (va_s, "Wv", "bv", 1.0)]:
            for tt in range(4):
                ps = PP.tile([128, Z], FP, tag="kv")
                nc.tensor.matmul(ps[:], S[W_n][:, ts(tt, 128)],
                                 S["aT"][:], start=True, stop=False)
                nc.tensor.matmul(ps[:], S[b_n][:, ts(tt, 128)],
                                 ones_r[:, 0:Z], start=False, stop=True)
                nc.scalar.activation(dst_s[:, tt, :], ps[:], AF.Copy,
                                     scale=scl)
        for tt in range(4):
            nc.scalar.mul(A0_s[:, tt, :], va_s[:, tt, :],
                          _fp(S["bgam1"][:, tt:tt + 1]))
        # amwT[z, f] = sum_g A0[g, z] * mW1[g, f], accumulated over 4 g-groups
        aps = PP.tile([128, HH], FP, tag="amwT")
        for j in range(4):
            nc.tensor.matmul(aps[:], A0_s[:, j, :], mW1_s[:, j, :],
                             start=(j == 0), stop=(j == 3))
        nc.scalar.copy(amwT_s[:], aps[:])
        # bqkT[z, h] = 0.125 * sum_f bq[f] k[f,z] [head(f)==h] (q-bias logits)
        bqk_ps = PP.tile([Z, NH], FP, tag="bqk")
        for tt in range(4):
            ek0 = PSB.tile([128, Z], FR, tag="ek0")
            nc.scalar.mul(ek0[:], kv_s[:, tt, :], _fp(S["bqc"][:, tt:tt + 1]))
            nc.tensor.matmul(bqk_ps[:], ek0[:], S["maskS"][:, ts(tt, NH)],
                             start=(tt == 0), stop=(tt == 3))
        nc.scalar.copy(bqkT_s[:], bqk_ps[:])

    # ---------------- B0/B1 under the ie_all pool ----------------
    with tc.tile_pool(name="iep", bufs=1) as ie_pool:
        ie_all = ie_pool.tile([128, CZALL], FR)
        # ---- B0: inv -> RFF -> ie (sin) ----
        with tc.tile_pool(name="b0_ps", bufs=2, space="PSUM") as PP, \
             tc.tile_pool(name="b0_jk", bufs=1, space="PSUM") as JP0, \
             tc.tile_pool(name="b0_sb", bufs=3) as SB:
            RC = 12582912.0  # 1.5 * 2^23: fp32 add rounds to nearest integer
            for i in range(NCHUNK):
                _warm(JP0, 3, ncols=384)
                cols = ts(i, CZ)
                inv = SB.tile([D, QC, Z], FR, tag="inv")
                nc.vector.tensor_sub(
                    inv[:], _bc_inner(_fp(S["xT"])[:, ts(i, QC)], Z),
                    _bc(_fp(S["pT"])[:, :], QC))
                # rows: [m_q, m_q+0.25, m_v, m_v+0.25] (unit-period phases)
                mm = PP.tile([128, CZ], FP, tag="mm")
                nc.tensor.matmul(mm[:], S["Bcat"][:], inv[:], start=True,
                                 stop=False)
                nc.tensor.matmul(mm[:], S["qb"][:], ones_r[:], start=False,
                                 stop=True)
                r1 = SB.tile([128, CZ], FP, tag="r1")
                nc.scalar.activation(r1[:], mm[:], AF.Copy, bias=RC)
                fr = SB.tile([128, CZ], FP, tag="fr")
                nc.vector.scalar_tensor_tensor(fr[:], r1[:], RC, mm[:],
                                               op0=OP.subtract,
                                               op1=OP.subtract)
                F = SB.tile([128, CZ], FR, tag="F")
                nc.scalar.activation(F[:], fr[:], AF.Sin,
                                     scale=float(2 * np.pi))
                ieps = PP.tile([128, CZ], FP, tag="ieps")
                nc.tensor.matmul(ieps[:], S["Wqv"][:], F[:],
                                 start=True, stop=True)
                nc.vector.tensor_copy(ie_all[:, cols], ieps[:])

        # ---- B1: q/logits, h1, vLN stats into PSUM (gelu) ----
        with tc.tile_pool(name="b1_st", bufs=1, space="PSUM") as PPS1:
            SvP = PPS1.tile([NCHUNK, CZ], FP)
            QvP = PPS1.tile([NCHUNK, CZ], FP)
            with tc.tile_pool(name="b1_ps", bufs=1, space="PSUM") as PP, \
                 tc.tile_pool(name="b1_qps", bufs=3, space="PSUM") as PPQ, \
                 tc.tile_pool(name="b1_jk", bufs=1, space="PSUM") as JP1, \
                 tc.tile_pool(name="b1_ek", bufs=8) as SBE, \
                 tc.tile_pool(name="b1_sb", bufs=2) as SB:
                for i in range(NCHUNK):
                    _warm(JP1, 2, ncols=256)
                    cols = ts(i, CZ)
                    # h1 path first: its gelu/square run on ACT while the PE
                    # works through the q-path matmuls below
                    h1ps = PP.tile([H, CZ], FP, tag="h1ps")
                    nc.tensor.matmul(h1ps[:], vW1_hi[:], ie_all[64:128, cols],
                                     start=True, stop=True)
                    nc.scalar.activation(h1_all[0:64, cols], h1ps[:], GELU_AF,
                                         bias=_fp(S["vb1p"])[:])
                    sq = SB.tile([H, CZ], FR, tag="sq")
                    nc.scalar.square(sq[:], _fp(h1_all[0:64, cols]))
                    qpss = []
                    for tt in range(4):
                        qps = PPQ.tile([128, CZ], FP, tag="qps")
                        nc.tensor.matmul(qps[:], S["Wq"][:, ts(tt, 128)],
                                         ie_all[0:64, cols],
                                         start=True, stop=True)
                        qpss.append(qps)
                    eks = []
                    for tt in range(4):
                        ek = SBE.tile([128, CZ], FR, tag="ek")
                        nc.vector.tensor_mul(ek[:], qpss[tt][:],
                                             _bc(kv_s[:, tt, :], QC))
                        eks.append(ek)
                    lps = PP.tile([NH, CZ], FP, tag="lps")
                    for tt in range(4):
                        nc.tensor.matmul(lps[:], S["maskS"][:, ts(tt, NH)],
                                         eks[tt][:], start=(tt == 0),
                                         stop=False)
                    nc.tensor.matmul(lps[:], bqkT_s[:], S["eyeZ"][:],
                                     start=False, stop=True)
                    nc.scalar.copy(mid_all[64:64 + NH, cols], lps[:])
                    sel = S["selS"][0:64, ts(i, NCHUNK)]
                    nc.tensor.matmul(SvP[:], sel, h1_all[0:64, cols],
                                     start=(i == 0), stop=(i == NCHUNK - 1))
                    nc.tensor.matmul(QvP[:], sel, sq[:],
                                     start=(i == 0), stop=(i == NCHUNK - 1))

            # ---- C1: vLN rstd (ln/exp); stats read from PSUM in place ----
            ln_math(nc, slice(0, NCHUNK), SvP, QvP, MvRv[:, 0, :], float(H),
                    False, MvRv[:, 1, :], eps_c)
            nc.sync.dma_start(t["scr_mr"], MvRv[:])

    # ---------------- quarters: B2 (gelu) -> ln+softmax -> D -------------
    h2_pool = stack.enter_context(tc.tile_pool(name="h2p", bufs=1))
    h2_q = h2_pool.tile([128, 4, CPQ * CZ], FR)
    P_bc = stack.enter_context(tc.tile_pool(name="bcast", bufs=8))

    def prefetch_mr(qq):
        """Broadcast each chunk's vLN (mean|rstd) row pair to H rows."""
        pf = []
        for ii in range(CPQ):
            i = qq * CPQ + ii
            mr = P_bc.tile([H, 2, CZ], FP, tag="mr")
            src = t["scr_mr"][i, :, :]
            nc.sync.dma_start(mr[:], bass.AP(tensor=src.tensor,
                                             offset=src.offset,
                                             ap=[[0, H]] + list(src.ap)))
            pf.append(mr)
        return pf

    pf_cur = prefetch_mr(0)
    for qq in range(QSPLIT):
        with tc.tile_pool(name="b2_st", bufs=1, space="PSUM") as PPS:
            SmP = PPS.tile([CPQ, CZ], FP, tag="SmP")
            QmP = PPS.tile([CPQ, CZ], FP, tag="QmP")
            with tc.tile_pool(name="qb_jk", bufs=1, space="PSUM") as JPQ:
                _warm(JPQ, 3)
            with tc.tile_pool(name="b2_pg", bufs=2, space="PSUM") as PPG, \
                 tc.tile_pool(name="b2_v1", bufs=4, space="PSUM") as PPV, \
                 tc.tile_pool(name="b2_s4", bufs=4) as SB4, \
                 tc.tile_pool(name="b2_sb", bufs=2) as SB, \
                 tc.tile_pool(name="b2_g4", bufs=7) as SBG:
                h1rs = []
                for ii in range(CPQ):
                    cols = ts(qq * CPQ + ii, CZ)
                    h1c = SB.tile([H, CZ], FP, tag="h1c")
                    nc.vector.tensor_sub(h1c[:], _fp(h1_all[0:64, cols]),
                                         pf_cur[ii][:, 0, :])
                    h1r = SB4.tile([H, CZ], FR, tag="h1r")
                    nc.vector.tensor_mul(h1r[:], h1c[:], pf_cur[ii][:, 1, :])
                    h1rs.append(h1r)
                for ii in range(CPQ):
                    qcols = ts(ii, CZ)
                    h1r = h1rs[ii]
                    Gs = []
                    for tt in range(4):
                        pg = PPG.tile([128, CZ], FP, tag="pg")
                        nc.tensor.matmul(pg[:], S["Wgam"][:, ts(tt, 128)],
                                         h1r[:], start=True, stop=True)
                        G = SBG.tile([128, CZ], FR, tag="G")
                        nc.vector.tensor_mul(G[:], _bc(va_s[:, tt, :], QC),
                                             pg[:])
                        Gs.append(G)
                    for dst in range(4):
                        v1p = PPV.tile([128, CZ], FP, tag="v1p")
                        for tt in range(4):
                            nc.tensor.matmul(v1p[:],
                                             mW1_s[:, tt, ts(dst, 128)],
                                             Gs[tt][:], start=(tt == 0),
                                             stop=False)
                        nc.tensor.matmul(v1p[:], S["Wbm"][:, ts(dst, 128)],
                                         h1r[:], start=False, stop=False)
                        nc.tensor.matmul(v1p[:], amwT_s[:, ts(dst, 128)],
                                         S["eyeZ"][:], start=False, stop=True)
                        nc.scalar.activation(h2_q[:, dst, qcols], v1p[:],
                                             GELU_AF,
                                             bias=_fp(S["mb1pp"])[:,
                                                                  dst:dst + 1])
                    # stats after the chunk's gelus
                    sel = S["selS"][:, ts(ii, NCHUNK)][:, 0:CPQ]
                    for dst in range(4):
                        nc.tensor.matmul(SmP[:], sel, h2_q[:, dst, qcols],
                                         start=(ii == 0 and dst == 0),
                                         stop=(ii == CPQ - 1 and dst == 3))
                        sq2 = SB.tile([128, CZ], FR, tag="sq2")
                        nc.scalar.square(sq2[:], _fp(h2_q[:, dst, qcols]))
                        nc.tensor.matmul(QmP[:], sel, sq2[:],
                                         start=(ii == 0 and dst == 0),
                                         stop=(ii == CPQ - 1 and dst == 3))

            # ---- mixer LN stats + per-quarter softmax (ln/exp table) ----
            # prefetch next quarter's mean/rstd rows ahead of the ln chain
            # so they don't queue behind it on the sync engine
            pf_next = prefetch_mr(qq + 1) if qq + 1 < QSPLIT else None
            qall = ts(qq, CPQ * CZ)
            ln_math(nc, slice(0, CPQ), SmP, QmP, nMq, float(HH), True, RmQ,
                    eps_c, mt_fr=True)
            nc.sync.dma_start(nMm1[:, :], nMq[0:CPQ, :])
            nc.sync.dma_start(t["scr_rm"][qq * CPQ:(qq + 1) * CPQ, :],
                              RmQ[0:CPQ, :])
            # unnormalized softmax: attention stays exp(logits); the 1/sum
            # is applied once to y_all right before the output projection
            attq = mid_all[64:64 + NH, qall]
            nc.scalar.activation(attq, _fp(attq), AF.Exp)
            esq = esum_all[64:64 + NH, ts(qq, CPQ * QC)]
            nc.vector.reduce_sum(
                esq, _fp(attq).rearrange("p (c z) -> p c z", z=Z), axis=AX.X)
            nc.vector.reciprocal(esq, esq)
            with tc.tile_pool(name="ln_jk", bufs=1, space="PSUM") as JPL:
                _warm(JPL, 3)

        # ---- D: v2, rank-1 mean fix, attention apply ----
        with tc.tile_pool(name="d_v2", bufs=5, space="PSUM") as PPV2, \
             tc.tile_pool(name="d_ab", bufs=2, space="PSUM") as PPA, \
             tc.tile_pool(name="d_jk", bufs=1, space="PSUM") as JPD, \
             tc.tile_pool(name="d_s4", bufs=4) as SD4, \
             tc.tile_pool(name="d_r2", bufs=4) as SDR, \
             tc.tile_pool(name="d_sb", bufs=2) as SB:
            rmss = []
            for ii in range(CPQ):
                i = qq * CPQ + ii
                rms8t = SDR.tile([64 + NH, CZ], FP, tag="rms8")
                nc.sync.dma_start(rms8t[64:64 + NH, :],
                                  _pbc(t["scr_rm"][i:i + 1, :], NH))
                rmss.append(rms8t)
            for ii in range(CPQ):
                _warm(JPD, 1, ncols=256)
                i = qq * CPQ + ii
                cols = ts(i, CZ)
                qcols = ts(ii, CZ)
                # mixer-LN rstd folded into the 8-row attention tile
                attRt = SD4.tile([64 + NH, CZ], FR, tag="attR")
                attR = attRt[64:64 + NH, :]
                nc.vector.tensor_mul(attR, _fp(mid_all[64:64 + NH, cols]),
                                     rmss[ii][64:64 + NH, :])
                for dst in range(4):
                    v2p = PPV2.tile([128, CZ], FP, tag="v2p")
                    for j in range(4):
                        nc.tensor.matmul(v2p[:], mW2_s[:, j, ts(dst, 128)],
                                         h2_q[:, j, qcols],
                                         start=(j == 0), stop=False)
                    nc.tensor.matmul(v2p[:], S["csmW2"][:, ts(dst, 128)],
                                     nMm1[:, qcols], start=False, stop=True)
                    ab = PPA.tile([128, CZ], FP, tag="ab")
                    nc.tensor.matmul(ab[:], maskB_hi[:, ts(dst, 128)],
                                     attR, start=True, stop=True)
                    abs_ = SB.tile([128, CZ], FP, tag="abs")
                    nc.scalar.copy(abs_[:], ab[:])
                    yp = SB.tile([128, QC, Z], FP, tag="yp")
                    nc.vector.tensor_mul(
                        yp[:], abs_[:].rearrange("p (c z) -> p c z", z=Z),
                        v2p[:].rearrange("p (c z) -> p c z", z=Z))
                    with nc.allow_low_precision(reason="fp32r y"):
                        nc.vector.reduce_sum(
                            y_all[:, dst, i * QC:(i + 1) * QC],
                            yp[:], axis=AX.X)
        pf_cur = pf_next

    # ---------------- OUT ----------------
    with tc.tile_pool(name="o_ps", bufs=1, space="PSUM") as PP, \
         tc.tile_pool(name="o_sb", bufs=1) as SB:
        # per-feature gather of the deferred softmax 1/sum, then normalize
        esY = PP.tile([128, 4 * CPC], FP)
        for tt in range(4):
            nc.tensor.matmul(esY[:, ts(tt, CPC)],
                             _fp(S["maskT"])[:, ts(tt, 128)],
                             esum_all[64:64 + NH, :], start=True, stop=True)
        y_n = SB.tile([128, 4, CPC], FR)
        nc.vector.tensor_mul(y_n[:], _fp(y_all[:]),
                             esY[:].rearrange("p (t c) -> p t c", t=4))
        ops = PP.tile([CPC, HH], FP)
        for j in range(4):
            nc.tensor.matmul(ops[:], y_n[:, j, :], Wo_s[:, j, :],
                             start=(j == 0), stop=False)
        nc.tensor.matmul(ops[:], ones_r[:, 0:CPC], S["bopp"][:],
                         start=False, stop=True)
        osb = SB.tile([CPC, HH], FP)
        nc.scalar.copy(osb[:], ops[:])
        nc.sync.dma_start(t["out"], osb[:])
    stack.close()


def ln_math(nc, rows, St, Qt, Mt, n, negate_mean, Rt, eps_c, mt_fr=False):
    # St/Qt may live in PSUM (DVE reads at most one PSUM input per op).
    # Mt = (+-)mean; Rt staged as scratch for S^2/n; Qt consumed in place.
    mt_rd = (lambda ap: _fp(ap)) if mt_fr else (lambda ap: ap)
    sgn = -1.0 if negate_mean else 1.0
    nc.vector.tensor_scalar_mul(Mt[rows, :], St[rows, :], sgn / n)
    nc.vector.tensor_mul(Rt[rows, :], St[rows, :], mt_rd(Mt[rows, :]))
    if negate_mean:
        nc.vector.tensor_add(Qt[rows, :], Qt[rows, :], Rt[rows, :])
    else:
        nc.vector.tensor_sub(Qt[rows, :], Qt[rows, :], Rt[rows, :])
    nc.scalar.activation(Qt[rows, :], Qt[rows, :], AF.Ln,
                         scale=1.0 / n, bias=eps_c[rows, :])
    nc.scalar.activation(Rt[rows, :], Qt[rows, :], AF.Exp, scale=-0.5)


# ======================= host side =======================
_CACHE = {}


def _pack_consts(P):
    A = np.zeros((128, CPK_NCOL), np.float32)
    for n, (r0, nr, c0, ncol) in CPK_COLS.items():
        if n in ("xp", "aT"):
            continue
        v = P[n]
        assert v.shape == (nr, ncol), (n, v.shape, nr, ncol)
        A[r0:r0 + nr, c0:c0 + ncol] = v
    return A


def _host_prep(inp):
    g = {k: np.ascontiguousarray(np.asarray(v, np.float32)) for k, v in inp.items()}
    P = {}
    P["Bcat"] = np.concatenate([g["B_q"], g["B_q"], g["B_v"], g["B_v"]], 1)
    qb = np.zeros((1, 128), np.float32)
    qb[0, 32:64] = 0.25
    qb[0, 96:128] = 0.25
    P["qb"] = qb
    Wqv = np.zeros((128, 128), np.float32)
    Wqv[0:64, 0:64] = -np.concatenate([g["Wqe"][:32], g["Wqe"][32:]], 0)
    Wqv[64:128, 64:128] = -np.concatenate([g["Wve"][:32], g["Wve"][32:]], 0)
    P["Wqv"] = Wqv
    bqp = (g["bqe"] @ g["Wq"] + g["bq"])[None, :]
    P["bqc"] = np.ascontiguousarray(bqp.reshape(4, 128).T)  # 0.125 is in kv_s
    P["vW1"] = g["vW1"]
    P["vb1p"] = (g["bve"] @ g["vW1"] + g["vb1"])[:, None]
    vW2p = g["vg"][:, None] * g["vW2"]
    vb2p = g["vbn"] @ g["vW2"] + g["vb2"]
    Wgam = vW2p[:, :HH]
    Wbeta, bbeta = vW2p[:, HH:], vb2p[HH:]
    bgam1 = np.ascontiguousarray((1.0 + vb2p[:HH]).reshape(4, 128).T)
    P["mW1"] = g["mW1"]
    Wbm = Wbeta @ g["mW1"]
    mb1pp = np.ascontiguousarray(
        (bbeta @ g["mW1"] + g["mb1"]).reshape(4, 128).T)
    mW2p = g["mg"][:, None] * g["mW2"]
    mb2p = g["mbn"] @ g["mW2"] + g["mb2"]
    P["mW2"] = mW2p
    csmW2 = mW2p.sum(0)[None, :]
    P["Wo"] = g["Wo"]
    bopp = (mb2p @ g["Wo"] + g["bo"])[None, :]
    P["Wcat"] = np.concatenate([g["Wq"], g["Wk"], g["Wv"], Wgam, Wbm], 1)
    P["brow"] = np.concatenate([bqp, g["bk"][None, :], g["bv"][None, :],
                                csmW2, bopp], 1)
    P["bcol"] = np.concatenate([bgam1, mb1pp], 1)
    for wn in ("mW1", "mW2", "Wo"):
        P[wn] = np.ascontiguousarray(
            P[wn].reshape(4, 128, HH).transpose(1, 0, 2).reshape(128, 4 * HH))
    P["onec"] = np.ones((128, 1), np.float32)
    P["oner"] = np.ones((1, CZ), np.float32)
    selS = np.zeros((128, NCHUNK, NCHUNK), np.float32)
    for i in range(NCHUNK):
        selS[:, i, i] = 1.0
    P["selS"] = np.ascontiguousarray(selS.reshape(128, NCHUNK * NCHUNK))
    P["eyeZ"] = np.ascontiguousarray(np.tile(np.eye(Z, dtype=np.float32),
                                             (1, QC)))
    mS = np.zeros((128, 4, NH), np.float32)
    for tt in range(4):
        for p in range(128):
            mS[p, tt, 2 * tt + p // 64] = 1.0
    P["maskS"] = np.ascontiguousarray(mS.reshape(128, 32))
    P["maskB"] = np.zeros((NH, HH), np.float32)
    for h in range(NH):
        P["maskB"][h, h * H:(h + 1) * H] = 1.0
    mT = np.zeros((NH, 4, 128), np.float32)
    for tt in range(4):
        for p in range(128):
            mT[2 * tt + p // 64, tt, p] = 1.0
    P["maskT"] = np.ascontiguousarray(mT.reshape(NH, 4 * 128))
    return P, g


def make_in_maps(P, g):
    base = _pack_consts(P)
    xT_full = np.ascontiguousarray(g["inputs"].reshape(B * C, D).T)
    in_maps = []
    for core in range(NCORE):
        b = core // (NCORE // B)
        A = base.copy()
        r0, nr, c0, ncol = CPK_COLS["xp"]
        A[r0:r0 + nr, c0:c0 + ncol] = np.concatenate(
            [xT_full[:, core * CPC:(core + 1) * CPC], g["p"][b].T], 1)
        r0, nr, c0, ncol = CPK_COLS["aT"]
        A[r0:r0 + nr, c0:c0 + ncol] = g["a"][b].T
        in_maps.append({"cpack": A})
    return in_maps


def kernel(**inputs):
    P, g = _host_prep(inputs)
    if "nc" not in _CACHE:
        _CACHE["nc"] = build_kernel()
    nc = _CACHE["nc"]
    in_maps = make_in_maps(P, g)
    res = run_bass_kernel_spmd(nc, in_maps, core_ids=list(range(NCORE)))
    outs = [res.results[i]["out"] for i in range(NCORE)]
    return np.concatenate(outs, 0).reshape(B, C, HH).astype(np.float32)


if __name__ == "__main__":
    import reference
    inp = {k: np.asarray(v) for k, v in reference.setup_inputs().items()}
    got = kernel(**inp)
    exp = np.asarray(reference.reference(**reference.setup_inputs()))
    err = np.abs(got - exp)
    scale = float(np.sqrt((exp ** 2).mean()))
    print("max abs err:", err.max(), " scaled:", err.max() / scale)
